# revision 33
# baseline (speedup 1.0000x reference)
"""Multi-level block-diagonal sparse attention (AttMLR) on 8 TRN2 NeuronCores.

Sharding: head-parallel — core c owns heads (2c, 2c+1). Each core:
  1. wave A: accumulates qT/kT ([d, t] layout, scaled) chunk-by-chunk as the
     replicated x^T streams in; wave B: vT + PE transposes to v ([t, d]).
  2. computes causal multi-level scores with per-tile causal-restricted
     streams; exp is split across engines: ACT (exact, with the 128/ln2
     pre-scale folded into Wq and undone via the activation scale) for
     diagonal tiles plus half the off-diagonal pairs, DVE one-op Schraudolph
     bit-exp (add + f32->int16 convert, bitcast as bf16) for the rest;
     triangular masks split DVE/gpsimd; y^T = v.T @ p^T accumulates with a
     fused ones-column giving the softmax denominator. Per q-block, diagonal
     tiles go FIRST (slow exact exps overlap the off-diag matmul run) and the
     final AV + normalize of block j is deferred until block j+1's first
     scores are emitted so the PE never stalls at block boundaries.
  3. each block j writes its halves into slots 2j/2j+1 of one AllToAll
     input; a single compact A2A after block 3 redistributes so core c holds
     all heads' dims for t-slice c; dummy ident matmuls bridge the collective
     wait; out_slice = y_slice @ Wproj with slot-granular accumulation.
Host assembles the 8 [256, 1024] f32 slices.

Level structure: RANKS [32, 16, 16] over head-dim prefixes with block sizes
[2048, 1024, 512]. Blocks nest, so a (k_tile, q_block) pair contracts over a
prefix of the 64 dims: 64 if same 512-block, 48 if same 1024-block, else 32.
Per-level 1/(rank*3) scaling (and the bit-exp pre-scale) is folded into Wq.
"""

import ml_dtypes
import numpy as np

import concourse.bass as bass
import concourse.mybir as mybir
from concourse import bacc
from concourse.bass_utils import run_bass_kernel_spmd
from concourse.tile import TileContext
from concourse.masks import make_identity

T = 2048
C = 1024
H = 16
D = 64
NCORES = 8
P = 128
NO = C // P          # 8 contraction chunks of 128
QB = 512             # q-block size (score-tile free dim)
NQB = T // QB        # 4 q-blocks
NKT = T // P         # 16 k-tiles
TS = T // NCORES     # 256, per-core output t-slice
F32 = mybir.dt.float32
BF16 = mybir.dt.bfloat16
I16 = mybir.dt.int16
NPBF16 = ml_dtypes.bfloat16
EXP = mybir.ActivationFunctionType.Exp
ADD = mybir.AluOpType.add

# Schraudolph bf16 exp: bf16_bits(e^s) ~= round(s*128/ln2 + (16256 - 5.5));
# the multiply is folded into Wq on the host, so scores arrive pre-scaled
# and the DVE pass is a single add + int16 convert. ACT undoes the scale.
EXP_A = 128.0 / float(np.log(2.0))
EXP_B = 16256.0 - 5.5

_CACHE = {}


def _ki(i, j):
    """Contraction depth for score tile (k_tile i, q_block j)."""
    if i // 4 == j:
        return 64
    if i // 8 == j // 2:
        return 48
    return 32


def _build():
    nc = bacc.Bacc(None, target_bir_lowering=False, num_devices=NCORES)

    xT = nc.declare_dram_parameter("xT", [P, NO, T], BF16, isOutput=False)
    wq = nc.declare_dram_parameter("wq", [P, NO, P], BF16, isOutput=False)
    wk = nc.declare_dram_parameter("wk", [P, NO, P], BF16, isOutput=False)
    wv = nc.declare_dram_parameter("wv", [P, NO, P], BF16, isOutput=False)
    wproj = nc.declare_dram_parameter("wproj", [P, NO, C], BF16, isOutput=False)
    tri = nc.declare_dram_parameter("tri", [P, P], BF16, isOutput=False)
    out = nc.declare_dram_parameter("out", [P, 2, C], F32, isOutput=True)

    with TileContext(nc) as tc:
        with (
            tc.tile_pool(name="persist", bufs=1) as persist,
            tc.tile_pool(name="pt", bufs=16) as ptp,
            tc.tile_pool(name="nrm", bufs=4) as nrm,
            tc.tile_pool(name="dram", bufs=1, space="DRAM") as dram,
        ):
            wq_sb = persist.tile([P, NO, P], BF16)
            wk_sb = persist.tile([P, NO, P], BF16)
            wv_sb = persist.tile([P, NO, P], BF16)
            wproj_sb = persist.tile([P, NO, C], BF16)
            tri_sb = persist.tile([P, P], BF16)
            ident = persist.tile([P, P], BF16)
            # chunked tensors -> fine-grained RAW deps
            xT_sb = [persist.tile([P, T], BF16, name=f"xT{o}") for o in range(NO)]
            qT_sb = [persist.tile([P, QB], BF16, name=f"qT{b}") for b in range(NQB)]
            kT_sb = [persist.tile([P, QB], BF16, name=f"kT{b}") for b in range(NQB)]
            vT_sb = [persist.tile([P, QB], BF16, name=f"vT{b}") for b in range(NQB)]
            # v in natural [t, d] layout; per t_tile a [128, 2, 65] whose last
            # column per head is 1.0 (softmax denominator row).
            v_sb = [persist.tile([P, 2, 65], BF16, name=f"v{i}") for i in range(NKT)]
            yT_sb = [persist.tile([P, QB], BF16, name=f"yT{b}") for b in range(NQB)]
            # received AllToAll slots: ys[s] = core s's heads for my t-slice
            ys_sb = [persist.tile([P, TS], BF16, name=f"ys{s}") for s in range(NCORES)]

            # input DMAs: xT has priority (phase 1 streams on it); weights on
            # gpsimd; wproj is deferred (issued on scalar after the wave-A
            # drains) so it cannot contend with the xT stream.
            nc.sync.dma_start(wk_sb[:], wk[:])
            for o in range(0, NO, 2):
                nc.sync.dma_start(xT_sb[o][:], xT[:, o, :])
                nc.scalar.dma_start(xT_sb[o + 1][:], xT[:, o + 1, :])
            nc.gpsimd.dma_start(wq_sb[:], wq[:])
            nc.gpsimd.dma_start(wv_sb[:], wv[:])
            nc.gpsimd.dma_start(tri_sb[:], tri[:])
            for i in range(NKT):
                nc.gpsimd.memset(v_sb[i][:, :, 64], 1.0)
            make_identity(nc, ident[:])
            a2a_in = dram.tile([NCORES, P, TS], BF16, name="a2ain")
            a2a_out = dram.tile([NCORES, P, TS], BF16, name="a2aout")

            # PE warmup gated on the wk DMA so it runs right before wave A
            # (HAM un-throttle) + ACT exp-table preload.
            with tc.tile_pool(name="warm", bufs=1, space="PSUM") as wps:
                wp = wps.tile([P, P], F32, tag="warm")
                for _ in range(24):
                    nc.tensor.matmul(wp[:], wk_sb[:, 0, :], ident[:],
                                     start=True, stop=True)
                wact = nrm.tile([1, 1], F32, tag="wact")
                nc.scalar.activation(wact[:], ident[0:1, 0:1], EXP)

            # ---- Phase 1 wave A: kT/qT accumulate chunk-by-chunk ----
            with tc.tile_pool(name="psA", bufs=1, space="PSUM") as psA:
                acc = {}
                for b in range(NQB):
                    acc[("k", b)] = psA.tile([P, QB], F32, tag=f"aK{b}",
                                             name=f"accK{b}")
                    acc[("q", b)] = psA.tile([P, QB], F32, tag=f"aQ{b}",
                                             name=f"accQ{b}")
                for o in range(NO):
                    for b in range(NQB):
                        for w, w_sb in (("k", wk_sb), ("q", wq_sb)):
                            nc.tensor.matmul(
                                acc[(w, b)][:],
                                w_sb[:, o, :],
                                xT_sb[o][:, bass.ts(b, QB)],
                                start=(o == 0),
                                stop=(o == NO - 1),
                            )
                # drain to SBUF; block 0 first so scores j=0 can start
                di = 0
                for b in range(NQB):
                    for w, dst in (("k", kT_sb[b]), ("q", qT_sb[b])):
                        if di % 2 == 0:
                            nc.vector.tensor_copy(dst[:], acc[(w, b)][:])
                        else:
                            nc.scalar.copy(dst[:], acc[(w, b)][:])
                        di += 1
            # wproj load starts only after the drains above (xT priority)
            nc.scalar.dma_start(wproj_sb[:], wproj[:])

            # ---- Phase 1 wave B: vT + transpose to v[t, d] ----
            # software-pipelined: block b+1's accumulation matmuls cover the
            # DVE copy latency that block b's transposes wait on, so the PE
            # stream stays gap-free (micro-idles re-throttle the HAM clock)
            with tc.tile_pool(name="psB", bufs=2, space="PSUM") as psB:
                def emit_vacc(b):
                    vacc = psB.tile([P, QB], F32, tag="vacc", name=f"vacc{b}")
                    for o in range(NO):
                        nc.tensor.matmul(
                            vacc[:],
                            wv_sb[:, o, :],
                            xT_sb[o][:, bass.ts(b, QB)],
                            start=(o == 0),
                            stop=(o == NO - 1),
                        )
                    nc.vector.tensor_copy(vT_sb[b][:], vacc[:])

                def emit_trans(b):
                    ptr = psB.tile([P, QB], BF16, tag="ptr", name=f"ptr{b}")
                    for q in range(4):
                        tt = 4 * b + q
                        nc.tensor.transpose(
                            ptr[:, bass.ts(q, P)],
                            vT_sb[b][:, bass.ts(q, P)],
                            ident[:],
                        )
                        nc.vector.tensor_copy(
                            v_sb[tt][:, :, 0:64],
                            ptr[:, bass.ts(q, P)].rearrange(
                                "p (h d) -> p h d", h=2
                            ),
                        )

                for b in range(NQB):
                    emit_vacc(b)
                    emit_trans(b)

            # ---- Phase 2: scores -> exp -> mask -> y^T accumulation ----
            with (
                tc.tile_pool(name="ps2s", bufs=3, space="PSUM") as ps2s,
                tc.tile_pool(name="ps2y", bufs=1, space="PSUM") as ps2y,
            ):
                offdiag_unit = 0  # alternates exact-ACT / DVE bit-exp

                def emit_scores_exp(j, pi, i0, i1):
                    nonlocal offdiag_unit
                    diag = i0 >= 4 * j
                    sps = [
                        ps2s.tile([P, 2 * QB], F32, tag="sps",
                                  name=f"sps{hh}_{j}_{pi}")
                        for hh in range(2)
                    ]
                    ptt = [
                        ptp.tile([P, 2 * QB], BF16, tag="pt",
                                 name=f"pt{hh}_{j}_{pi}")
                        for hh in range(2)
                    ]
                    for half, i in enumerate((i0, i1)):
                        ki = _ki(i, j)
                        lo = P * (i % 4) if diag else 0  # causal col start
                        for h in range(2):
                            nc.tensor.matmul(
                                sps[h][:, half * QB + lo : (half + 1) * QB],
                                kT_sb[i // 4][h * D : h * D + ki,
                                              bass.ts(i % 4, P)],
                                qT_sb[j][h * D : h * D + ki, lo:QB],
                                start=True,
                                stop=True,
                                tile_position=(h * D, 0),
                            )
                    if diag:
                        for h in range(2):
                            for half, i in enumerate((i0, i1)):
                                lo = P * (i % 4)
                                nc.scalar.activation(
                                    ptt[h][:, half * QB + lo : (half + 1) * QB],
                                    sps[h][:, half * QB + lo : (half + 1) * QB],
                                    EXP,
                                    scale=1.0 / EXP_A,
                                )
                        # triangular mask on the boundary strip (DVE; gpsimd
                        # is kept to broadcasts only — putting masks on its
                        # queue measurably serializes the AV chain)
                        for h in range(2):
                            for half, i in enumerate((i0, i1)):
                                lo = P * (i % 4)
                                nc.vector.tensor_mul(
                                    ptt[h][:, half * QB + lo : half * QB + lo + P],
                                    ptt[h][:, half * QB + lo : half * QB + lo + P],
                                    tri_sb[:],
                                )
                    else:
                        with nc.allow_low_precision(reason="bitexp bf16"):
                            for h in range(2):
                                if offdiag_unit % 12 < 7:
                                    nc.scalar.activation(
                                        ptt[h][:], sps[h][:], EXP,
                                        scale=1.0 / EXP_A,
                                    )
                                else:
                                    nc.vector.tensor_scalar(
                                        ptt[h][:].bitcast(I16),
                                        sps[h][:],
                                        EXP_B,
                                        None,
                                        ADD,
                                    )
                                offdiag_unit += 1
                    return ptt

                def emit_av(j, yps, ptt, pair, diag, last):
                    for h in range(2):
                        for half, i in enumerate(pair):
                            lo = P * (i % 4) if diag else 0
                            nc.tensor.matmul(
                                yps[h][:, lo:QB],
                                v_sb[i][:, h, :],
                                ptt[h][:, half * QB + lo : (half + 1) * QB],
                                start=(i == 4 * j),
                                stop=(last and half == 1 and h == 1),
                            )

                def emit_finalize(j, yps):
                    for h in range(2):
                        # ACT copy releases the PSUM bank fast; the rest of
                        # the chain runs from SBUF (gpsimd: broadcast only)
                        yn = nrm.tile([65, QB], F32, tag="yn", name=f"yn{h}_{j}")
                        nc.scalar.copy(yn[:], yps[h][:])
                        den = nrm.tile([1, QB], F32, tag="den", name=f"den{h}_{j}")
                        nc.vector.tensor_copy(den[:], yn[64:65, :])
                        rec = nrm.tile([1, QB], F32, tag="rec", name=f"rec{h}_{j}")
                        nc.vector.reciprocal_approx_fast(rec[:], den[:])
                        bc = nrm.tile([64, QB], F32, tag="bc", name=f"bc{h}_{j}")
                        nc.gpsimd.partition_broadcast(bc[:], rec[:])
                        with nc.allow_low_precision(reason="bf16 y for comms"):
                            nc.vector.tensor_mul(
                                yT_sb[j][h * D : (h + 1) * D, :],
                                yn[0:64, :],
                                bc[:],
                            )
                    for half in range(2):
                        nc.sync.dma_start(
                            a2a_in[2 * j + half],
                            yT_sb[j][:, bass.ts(half, TS)],
                        )

                # 2-pair lookahead: AV for pair p is emitted only after the
                # scores of pairs p+1, p+2 are in the PE stream, so the PE
                # never head-of-line blocks on a pair's exp/mask chain.
                pending = []  # (j, yps, ptt, pair, diag, last_of_j)

                def pop_pending():
                    pj, pyps, pptt, ppair, pdiag, plast = pending.pop(0)
                    emit_av(pj, pyps, pptt, ppair, pdiag, last=plast)
                    if plast:
                        emit_finalize(pj, pyps)

                for j in range(NQB):
                    nkt = 4 * j + 4
                    order = list(range(4 * j, nkt)) + list(range(0, 4 * j))
                    pairs = [(order[2 * p], order[2 * p + 1])
                             for p in range(nkt // 2)]
                    yps = [
                        ps2y.tile([65, QB], F32, tag=f"yps{h}", name=f"yps{h}_{j}")
                        for h in range(2)
                    ]
                    for pi, (i0, i1) in enumerate(pairs):
                        ptt = emit_scores_exp(j, pi, i0, i1)
                        pending.append((j, yps, ptt, (i0, i1), i0 >= 4 * j,
                                        pi == len(pairs) - 1))
                        while len(pending) > 2:
                            pop_pending()
                while pending:
                    pop_pending()

            # single compact AllToAll once every block's slots are written:
            # slot s = yT[s//2][:, (s%2)-half]; core c receives, in out-slot
            # s', core s''s heads (C-dims 128s'..) for its own t-slice.
            nc.gpsimd.collective_compute(
                "AllToAll",
                mybir.AluOpType.bypass,
                replica_groups=[list(range(NCORES))],
                ins=[a2a_in.opt()],
                outs=[a2a_out.opt()],
            )
            issuers = (nc.sync, nc.scalar, nc.gpsimd)
            for s in range(NCORES):
                issuers[s % 3].dma_start(ys_sb[s][:], a2a_out[s])

            # dummy warm matmuls bridge the collective wait; anchored on the
            # last block's yT so the Tile scheduler cannot hoist them earlier
            with tc.tile_pool(name="warm2", bufs=1, space="PSUM") as wps2:
                wp2 = wps2.tile([P, P], F32, tag="warm2")
                for _ in range(150):
                    nc.tensor.matmul(wp2[:], yT_sb[3][:, 0:P], ident[:],
                                     start=True, stop=True)

            # ---- Phase 4: out_slice = y_slice @ Wproj ----
            with (
                tc.tile_pool(name="ps4", bufs=4, space="PSUM") as ps4,
                tc.tile_pool(name="st4", bufs=4) as st4,
            ):
                for tt in range(2):
                    for nb in range(2):
                        pso = ps4.tile([P, QB], F32, tag="pso",
                                       name=f"pso{tt}_{nb}")
                        for s in range(NCORES):
                            nc.tensor.matmul(
                                pso[:],
                                ys_sb[s][:, bass.ts(tt, P)],
                                wproj_sb[:, s, bass.ts(nb, QB)],
                                start=(s == 0),
                                stop=(s == NCORES - 1),
                            )
                        stage = st4.tile([P, QB], F32, tag="stage",
                                         name=f"stage{tt}_{nb}")
                        if nb == 0:
                            nc.scalar.copy(stage[:], pso[:])
                        else:
                            nc.vector.tensor_copy(stage[:], pso[:])
                        nc.scalar.dma_start(out[:, tt, bass.ts(nb, QB)], stage[:])

    nc.compile()
    return nc


def _prep_inputs(x, Wqkv, Wproj):
    x2 = np.ascontiguousarray(x.reshape(T, C))
    xT = np.ascontiguousarray(x2.T)                       # [C, T]
    xT_a = np.ascontiguousarray(
        xT.reshape(NO, P, T).transpose(1, 0, 2)
    ).astype(NPBF16)

    # per-dim scale folded into Wq: 1/(rank*3) by level of (d % 64), plus
    # the bit-exp pre-scale 128/ln2 (ACT divides it back out)
    colscale = np.where(np.arange(P) % D < 32, 1.0 / 96, 1.0 / 48).astype(
        np.float32
    ) * np.float32(EXP_A)

    wproj_a = np.ascontiguousarray(
        Wproj.reshape(NO, P, C).transpose(1, 0, 2)
    ).astype(NPBF16)

    kp = np.arange(P)[:, None]
    qf = np.arange(P)[None, :]
    tri_a = np.ascontiguousarray((qf >= kp).astype(np.float32)).astype(NPBF16)

    in_maps = []
    for c in range(NCORES):
        cs = slice(P * c, P * (c + 1))
        wq_c = Wqkv[:, cs] * colscale[None, :]
        wk_c = Wqkv[:, C : 2 * C][:, cs]
        wv_c = Wqkv[:, 2 * C :][:, cs]
        in_maps.append(
            {
                "xT": xT_a,
                "wq": np.ascontiguousarray(
                    wq_c.reshape(NO, P, P).transpose(1, 0, 2)
                ).astype(NPBF16),
                "wk": np.ascontiguousarray(
                    wk_c.reshape(NO, P, P).transpose(1, 0, 2)
                ).astype(NPBF16),
                "wv": np.ascontiguousarray(
                    wv_c.reshape(NO, P, P).transpose(1, 0, 2)
                ).astype(NPBF16),
                "wproj": wproj_a,
                "tri": tri_a,
            }
        )
    return in_maps


def kernel(x, Wqkv, Wproj, _trace=False):
    x = np.asarray(x, np.float32)
    Wqkv = np.asarray(Wqkv, np.float32)
    Wproj = np.asarray(Wproj, np.float32)

    if "nc" not in _CACHE:
        _CACHE["nc"] = _build()
    nc = _CACHE["nc"]

    in_maps = _prep_inputs(x, Wqkv, Wproj)
    res = run_bass_kernel_spmd(nc, in_maps, list(range(NCORES)), trace=_trace)
    _CACHE["last_result"] = res

    full = np.empty((T, C), np.float32)
    for c in range(NCORES):
        oc = res.results[c]["out"]  # [128, 2, 1024]
        full[2 * P * c : 2 * P * (c + 1)] = oc.transpose(1, 0, 2).reshape(
            2 * P, C
        )
    return full.reshape(1, T, C)
